# revision 14
# baseline (speedup 1.0000x reference)
"""CTBG circuit kernel for Trainium2, data-parallel over batch on 8 NeuronCores.

Network (per reference):
  gpe_out = x @ (gpe_w * gpe_mask.T) + gpe_b              [B, 1536]
  gpi_in  = concat([x, gpe_out], -1)                      [B, 3072]
  gpi_out = gpi_in @ (gpi_w * gpi_mask.T) + gpi_b         [B, 3072] @ [3072, 1536]
  h1 = relu(gpi_out @ w1 + b1); h2 = relu(h1 @ w2 + b2)
  out = relu(h2 @ w3 + b3)                                [B, 6]

Key algebraic identity: gpe_out and gpi_out feed forward with no
intervening nonlinearity, so the masked front end folds into one
[1536, 512] weight computed ON DEVICE once per launch:

  A  = gpe_w * gpe_mask.T          [1536 i, 1536 u]
  Bx = (gpi_w * gpi_mask.T)[:1536] [1536 i, 1536 v]
  Bu = (gpi_w * gpi_mask.T)[1536:] [1536 u, 1536 v]
  Wfold = Bx @ w1 + A @ (Bu @ w1)  [1536, 512]
  bfold = gpe_b @ (Bu @ w1) + gpi_b @ w1 + b1
  h1 = relu(x @ Wfold + bfold) -> h2 -> out   (per batch row)

Distribution: a fixed ~36us collectives-init barrier on this platform
gates the FIRST collective completion to ~90us into the launch, so
chained collectives (gather M, then gather Wfold) are poison.  Instead
every core computes a full-shape PARTIAL of Wfold from purely local
slices, and ONE AllReduce(add) sums them:

  core c:  M_c = Bu[usl_c] @ w1                 [192, 512]  (local)
           P_c = Bx[:, vsl_c] @ w1[vsl_c]       [1536, 512] (partial
               + A[:, usl_c] @ M_c                            sums)
           prow_c = gpe_b[usl_c] @ M_c + gpi_b[vsl_c] @ w1[vsl_c]
  AllReduce over cores: Wfold = sum_c P_c ; bias row = sum_c prow_c.

The AllReduce is split into two h-halves so the batch pass starts on
h-columns 0:256 while the second half is still on the wire.  The
batch pass keeps the stationary operand fixed across the 4 batch
tiles (i-outer, t-inner) to amortize LDWEIGHTS, 4 PSUM banks wide.
gpsimd queue carries only collectives; sync carries the
dependency-laden loads; scalar carries free-flowing streams (x).

Host prep is layout/dtype only (no FLOPs): bf16 casts, transposes of
x/gpe_w/gpi_w, row/column slicing and an even/odd interleave
permutation of each 192-row slice (so the two 96-row PE groups are
contiguous and drains are single DMAs).
"""

import numpy as np
import ml_dtypes

BF = ml_dtypes.bfloat16

NCORES = 8
B = 16384
BS = B // NCORES          # 2048 rows per core
BT = 512                  # batch tile (matmul free dim)
NBT = BS // BT            # 4
D1 = 1536                 # gpe input dim (x features)
H = 512                   # mlp hidden
HH = H // 2               # 256: AllReduce column half
A = 6                     # action dim
SL = D1 // NCORES         # 192: fold rows per core
HSL = SL // 2             # 96: interleaved half-slice
PR = D1 + 1               # AllReduce rows: 1536 Wfold + 1 bias row

NI = D1 // 128            # 12 i-chunks (x features)
NV = D1 // 128            # 12 v-chunks (gpi outputs)
NH = H // 128             # 4 h-chunks (mlp hidden)

_CACHE = {}


def _build():
    import concourse.bacc as bacc
    import concourse.tile as tile
    from concourse import mybir
    from concourse.masks import make_identity

    FP32 = mybir.dt.float32
    BF16 = mybir.dt.bfloat16
    Act = mybir.ActivationFunctionType

    nc = bacc.Bacc(None, num_devices=NCORES)

    xT_d = nc.dram_tensor("xT", [D1, BS], BF16, kind="ExternalInput")
    # [1536, 384] = [masked uslice cols | wT uslice cols], interleave-permuted
    gpiu_d = nc.dram_tensor("gpiu", [D1, 2 * SL], BF16, kind="ExternalInput")
    # [192, 3072] = [mask | wT] rows vsl (BxT) / usl (AT), interleave-permuted
    bxp_d = nc.dram_tensor("bxp", [SL, 2 * D1], BF16, kind="ExternalInput")
    ap_d = nc.dram_tensor("apk", [SL, 2 * D1], BF16, kind="ExternalInput")
    w1_d = nc.dram_tensor("w1", [D1, H], BF16, kind="ExternalInput")
    w1vs_d = nc.dram_tensor("w1vs", [SL, H], BF16, kind="ExternalInput")
    w2_d = nc.dram_tensor("w2", [H, H], BF16, kind="ExternalInput")
    w3_d = nc.dram_tensor("w3", [H, A], BF16, kind="ExternalInput")
    gpebp_d = nc.dram_tensor("gpebp", [HSL, 2], FP32, kind="ExternalInput")
    gpibp_d = nc.dram_tensor("gpibp", [HSL, 2], FP32, kind="ExternalInput")
    b1_d = nc.dram_tensor("b1", [H], FP32, kind="ExternalInput")
    b2_d = nc.dram_tensor("b2", [H], FP32, kind="ExternalInput")
    b3_d = nc.dram_tensor("b3", [A], FP32, kind="ExternalInput")
    o_d = nc.dram_tensor("out", [A, BS], FP32, kind="ExternalOutput")

    RG = [list(range(NCORES))]

    with tile.TileContext(nc) as tc:
        with (
            tc.tile_pool(name="wp", bufs=1) as wp,           # persistent
            tc.tile_pool(name="pc", bufs=1) as pcp,          # P drain staging
            tc.tile_pool(name="ap", bufs=1) as ap,           # activations
            tc.tile_pool(name="dp", bufs=1, space="DRAM") as dp,
            tc.tile_pool(name="psp", bufs=8, space="PSUM") as psp,
        ):
            def ps_tile():
                return psp.tile([128, BT], FP32, tag="ps", name="ps")

            # ---- fold operand stream: gpi uslice + w1 (F1s), alternating
            gpiu = []
            w1t = []
            for v in range(NV):
                q = nc.sync if (v % 2 == 0) else nc.scalar
                m = wp.tile([128, 2 * SL], BF16, tag=f"gpiu{v}")
                q.dma_start(out=m[:, :], in_=gpiu_d[v * 128:(v + 1) * 128, :])
                w = wp.tile([128, H], BF16, tag=f"w1_{v}")
                q.dma_start(out=w[:, :], in_=w1_d[v * 128:(v + 1) * 128, :])
                nc.vector.tensor_mul(m[:, 0:SL], m[:, 0:SL], m[:, SL:2 * SL])
                gpiu.append(m)
                w1t.append(w)

            # ---- partial-P operands: BxT/AT row-slices, w1 row-slice
            # big mask-multiplies on gpsimd (it idles until the AR triggers)
            # so DVE stays free to pace the fold drains
            bxp, apk, w1vs = [], [], []
            for g in range(2):
                t = wp.tile([HSL, 2 * D1], BF16, tag=f"bxp{g}")
                nc.sync.dma_start(out=t[:, :],
                                  in_=bxp_d[g * HSL:(g + 1) * HSL, :])
                nc.gpsimd.tensor_mul(t[:, 0:D1], t[:, 0:D1], t[:, D1:2 * D1])
                bxp.append(t)
                t = wp.tile([HSL, 2 * D1], BF16, tag=f"apk{g}")
                nc.scalar.dma_start(out=t[:, :],
                                    in_=ap_d[g * HSL:(g + 1) * HSL, :])
                nc.gpsimd.tensor_mul(t[:, 0:D1], t[:, 0:D1], t[:, D1:2 * D1])
                apk.append(t)
                t = wp.tile([HSL, H], BF16, tag=f"w1vs{g}")
                nc.sync.dma_start(out=t[:, :],
                                  in_=w1vs_d[g * HSL:(g + 1) * HSL, :])
                w1vs.append(t)

            # small loads
            gpebp = wp.tile([HSL, 2], FP32, tag="gpebp")
            nc.scalar.dma_start(out=gpebp[:, :], in_=gpebp_d[:, :])
            gpibp = wp.tile([HSL, 2], FP32, tag="gpibp")
            nc.scalar.dma_start(out=gpibp[:, :], in_=gpibp_d[:, :])
            gpebf = wp.tile([HSL, 2], BF16, tag="gpebf")
            nc.vector.tensor_copy(gpebf[:, :], gpebp[:, :])
            gpibf = wp.tile([HSL, 2], BF16, tag="gpibf")
            nc.vector.tensor_copy(gpibf[:, :], gpibp[:, :])
            b2_sb = wp.tile([128, NH], FP32, tag="b2sb")
            nc.scalar.dma_start(out=b2_sb[:, :],
                                in_=b2_d.rearrange("(c p) -> p c", p=128))
            b3_sb = wp.tile([A, 1], FP32, tag="b3sb")
            nc.scalar.dma_start(out=b3_sb[:, :],
                                in_=b3_d.rearrange("(a one) -> a one", one=1))
            b1row = wp.tile([1, H], FP32, tag="b1row")
            nc.scalar.dma_start(out=b1row[:, :],
                                in_=b1_d.rearrange("(one h) -> one h", one=1))
            w2t = []
            for k in range(NH):
                t = wp.tile([128, H], BF16, tag=f"w2_{k}")
                nc.scalar.dma_start(out=t[:, :], in_=w2_d[k * 128:(k + 1) * 128, :])
                w2t.append(t)
            w3t = []
            for k in range(NH):
                t = wp.tile([128, A], BF16, tag=f"w3_{k}")
                nc.scalar.dma_start(out=t[:, :], in_=w3_d[k * 128:(k + 1) * 128, :])
                w3t.append(t)
            ident = wp.tile([128, 128], FP32, tag="ident")
            make_identity(nc, ident[:, :])

            # ---- F1s: local M slice, two interleaved 96-row groups ->
            # msb[:, g*512:(g+1)*512] holds M rows {2p+g} in bf16
            ps_m = [ps_tile() for _ in range(2)]
            for v in range(NV):
                for g in range(2):
                    nc.tensor.matmul(ps_m[g][0:HSL, :],
                                     gpiu[v][:, g * HSL:(g + 1) * HSL],
                                     w1t[v][:, :],
                                     start=(v == 0), stop=(v == NV - 1))
            msb = wp.tile([HSL, 2 * H], BF16, tag="msb")
            for g in range(2):
                nc.vector.tensor_copy(msb[:, g * H:(g + 1) * H],
                                      ps_m[g][0:HSL, :])

            # ---- partial P chunks, drained pair-wise: two 128-row chunks
            # share one [128, 1024] staging tile, stored as one big DMA per
            # column half (a-halves on sync, b-halves on scalar)
            pa_dram = dp.tile([PR, HH], BF16, tag="pa_d")
            pb_dram = dp.tile([PR, HH], BF16, tag="pb_d")
            for ip in range(NI // 2):
                sb = pcp.tile([128, 2 * BT], BF16, tag=f"pcs{ip}")
                for ci in range(2):
                    i = 2 * ip + ci
                    ps = ps_tile()
                    for g in range(2):
                        nc.tensor.matmul(ps[:, :],
                                         bxp[g][:, i * 128:(i + 1) * 128],
                                         w1vs[g][:, :],
                                         start=(g == 0), stop=False)
                    for g in range(2):
                        nc.tensor.matmul(ps[:, :],
                                         apk[g][:, i * 128:(i + 1) * 128],
                                         msb[:, g * H:(g + 1) * H],
                                         start=False, stop=(g == 1))
                    nc.vector.tensor_copy(sb[:, ci * BT:(ci + 1) * BT],
                                          ps[:, :])
                sbv = sb[:, :].rearrange("p (c q) -> p c q", c=2)
                nc.sync.dma_start(
                    out=pa_dram[ip * 256:(ip + 1) * 256, :].rearrange(
                        "(c p) h -> p c h", c=2),
                    in_=sbv[:, :, 0:HH])
                nc.scalar.dma_start(
                    out=pb_dram[ip * 256:(ip + 1) * 256, :].rearrange(
                        "(c p) h -> p c h", c=2),
                    in_=sbv[:, :, HH:2 * HH])

            # bias partial row (row 1536 of the AllReduce payload)
            psb = ps_tile()
            for g in range(2):
                nc.tensor.matmul(psb[0:1, :], gpibf[:, g:g + 1], w1vs[g][:, :],
                                 start=(g == 0), stop=False)
            for g in range(2):
                nc.tensor.matmul(psb[0:1, :], gpebf[:, g:g + 1],
                                 msb[:, g * H:(g + 1) * H],
                                 start=False, stop=(g == 1))
            prow = wp.tile([1, H], BF16, tag="prow")
            nc.vector.tensor_copy(prow[:, :], psb[0:1, :])
            nc.sync.dma_start(out=pa_dram[D1:PR, :], in_=prow[:, 0:HH])
            nc.scalar.dma_start(out=pb_dram[D1:PR, :], in_=prow[:, HH:2 * HH])

            # ---- ONE AllReduce, split into two h-halves
            wfa_dram = dp.tile([PR, HH], BF16, tag="wfa_d", addr_space="Shared")
            wfb_dram = dp.tile([PR, HH], BF16, tag="wfb_d", addr_space="Shared")
            nc.gpsimd.collective_compute(
                "AllReduce", mybir.AluOpType.add, replica_groups=RG,
                ins=[pa_dram[:, :].opt()], outs=[wfa_dram[:, :].opt()])
            nc.gpsimd.collective_compute(
                "AllReduce", mybir.AluOpType.add, replica_groups=RG,
                ins=[pb_dram[:, :].opt()], outs=[wfb_dram[:, :].opt()])

            # ---- x tiles stream on scalar meanwhile
            xt = [[None] * NI for _ in range(NBT)]
            for t_i in range(NBT):
                for i in range(NI):
                    t = wp.tile([128, BT], BF16, tag=f"x{t_i}_{i}")
                    nc.scalar.dma_start(out=t[:, :],
                                        in_=xT_d[i * 128:(i + 1) * 128,
                                                 t_i * BT:(t_i + 1) * BT])
                    xt[t_i][i] = t

            # ---- Wfold reloads: bias row first, then a-halves (gated on
            # AR a), then b-halves
            browb = wp.tile([1, H], BF16, tag="browb")
            nc.sync.dma_start(out=browb[:, 0:HH], in_=wfa_dram[D1:PR, :])
            Wf = []
            for i in range(NI):
                t = wp.tile([128, H], BF16, tag=f"Wf{i}")
                nc.sync.dma_start(out=t[:, 0:HH],
                                  in_=wfa_dram[i * 128:(i + 1) * 128, :])
                Wf.append(t)
            nc.sync.dma_start(out=browb[:, HH:H], in_=wfb_dram[D1:PR, :])
            for i in range(NI):
                nc.sync.dma_start(out=Wf[i][:, HH:H],
                                  in_=wfb_dram[i * 128:(i + 1) * 128, :])

            # bias row + b1, transposed [1,512] -> [128,4] columns on the PE
            # (idle right after each AR half lands), per half so hc 0/1
            # activations don't wait on AR b.
            brow = wp.tile([1, H], FP32, tag="brow")
            bfold = wp.tile([128, NH], FP32, tag="bfold")

            def bias_half(half):
                lo, hi = half * HH, (half + 1) * HH
                nc.vector.tensor_add(brow[:, lo:hi], browb[:, lo:hi],
                                     b1row[:, lo:hi])
                for c in range(2 * half, 2 * half + 2):
                    pst = ps_tile()
                    nc.tensor.transpose(pst[:, 0:1],
                                        brow[0:1, c * 128:(c + 1) * 128],
                                        ident[0:1, 0:1])
                    nc.vector.tensor_copy(bfold[:, c:c + 1], pst[:, 0:1])

            bias_half(0)

            # ---- batch pass: hc 0/1 across all 4 batch tiles (gated on AR a
            # only), then per-tile [hc2, hc3, L2, L3 + store] so each output
            # store trails its own tile instead of the whole batch
            h1 = [[None] * NH for _ in range(NBT)]
            for hc in range(2):
                ps1 = [ps_tile() for _ in range(NBT)]
                for i in range(NI):
                    for t_i in range(NBT):
                        nc.tensor.matmul(ps1[t_i][:, :],
                                         Wf[i][:, hc * 128:(hc + 1) * 128],
                                         xt[t_i][i][:, :],
                                         start=(i == 0), stop=(i == NI - 1))
                for t_i in range(NBT):
                    h = ap.tile([128, BT], BF16, tag=f"h1_{t_i}_{hc}")
                    nc.scalar.activation(h[:, :], ps1[t_i][:, :], Act.Relu,
                                         bias=bfold[:, hc:hc + 1])
                    h1[t_i][hc] = h

            bias_half(1)

            for t_i in range(NBT):
                for hc in range(2, NH):
                    ps1 = ps_tile()
                    for i in range(NI):
                        nc.tensor.matmul(ps1[:, :],
                                         Wf[i][:, hc * 128:(hc + 1) * 128],
                                         xt[t_i][i][:, :],
                                         start=(i == 0), stop=(i == NI - 1))
                    h = ap.tile([128, BT], BF16, tag=f"h1_{t_i}_{hc}")
                    nc.scalar.activation(h[:, :], ps1[:, :], Act.Relu,
                                         bias=bfold[:, hc:hc + 1])
                    h1[t_i][hc] = h

                h2 = []
                for mc in range(NH):
                    ps2 = ps_tile()
                    for k in range(NH):
                        nc.tensor.matmul(ps2[:, :],
                                         w2t[k][:, mc * 128:(mc + 1) * 128],
                                         h1[t_i][k][:, :],
                                         start=(k == 0), stop=(k == NH - 1))
                    h = ap.tile([128, BT], BF16, tag=f"h2_{t_i}_{mc}")
                    nc.scalar.activation(h[:, :], ps2[:, :], Act.Relu,
                                         bias=b2_sb[:, mc:mc + 1])
                    h2.append(h)

                pso = ps_tile()
                for k in range(NH):
                    nc.tensor.matmul(pso[0:A, :], w3t[k][:, :], h2[k][:, :],
                                     start=(k == 0), stop=(k == NH - 1))
                osb = ap.tile([A, BT], FP32, tag=f"osb{t_i}")
                nc.scalar.activation(osb[:, :], pso[0:A, :], Act.Relu,
                                     bias=b3_sb[:, 0:1])
                nc.sync.dma_start(out=o_d[:, t_i * BT:(t_i + 1) * BT],
                                  in_=osb[:, :])

    nc.finalize()
    return nc


def _get_nc():
    if "nc" not in _CACHE:
        _CACHE["nc"] = _build()
    return _CACHE["nc"]


def _prep_inputs(inputs):
    """Host-side layout/dtype prep only (no network FLOPs): bf16 casts,
    transposes, per-core row/column slicing and interleave permutation."""
    f = {k: np.asarray(v) for k, v in inputs.items()}
    xT = np.ascontiguousarray(f["x"].astype(BF).T)            # [1536, B]
    gpem = f["gpe_mask"].astype(BF)                           # [u, i]
    gpewT = np.ascontiguousarray(f["gpe_w"].astype(BF).T)     # [u, i]
    gpim = f["gpi_mask"].astype(BF)                           # [v, j]
    gpiwT = np.ascontiguousarray(f["gpi_w"].astype(BF).T)     # [v, j]
    w1 = f["w1"].astype(BF)
    gpe_b = np.asarray(f["gpe_b"], dtype=np.float32)
    gpi_b = np.asarray(f["gpi_b"], dtype=np.float32)
    # packed position g*96 + p  <->  logical slice index 2p + g
    perm = np.concatenate([np.arange(0, SL, 2), np.arange(1, SL, 2)])
    shared = {
        "w1": np.ascontiguousarray(w1),
        "w2": np.ascontiguousarray(f["w2"].astype(BF)),
        "w3": np.ascontiguousarray(f["w3"].astype(BF)),
        "b1": np.ascontiguousarray(f["b1"], dtype=np.float32),
        "b2": np.ascontiguousarray(f["b2"], dtype=np.float32),
        "b3": np.ascontiguousarray(f["b3"], dtype=np.float32),
    }
    in_maps = []
    for c in range(NCORES):
        sl = np.arange(c * SL, (c + 1) * SL)[perm]   # permuted local slice
        usl = D1 + sl                                # gpi columns for u-part
        in_maps.append(dict(
            shared,
            xT=np.ascontiguousarray(xT[:, c * BS:(c + 1) * BS]),
            gpiu=np.ascontiguousarray(
                np.concatenate([gpim[:, usl], gpiwT[:, usl]], axis=1)),
            bxp=np.ascontiguousarray(
                np.concatenate([gpim[sl][:, :D1], gpiwT[sl][:, :D1]], axis=1)),
            apk=np.ascontiguousarray(
                np.concatenate([gpem[sl], gpewT[sl]], axis=1)),
            w1vs=np.ascontiguousarray(w1[sl]),
            gpebp=np.ascontiguousarray(
                gpe_b[sl].reshape(2, HSL).T, dtype=np.float32),
            gpibp=np.ascontiguousarray(
                gpi_b[sl].reshape(2, HSL).T, dtype=np.float32),
        ))
    return in_maps


def _run(inputs, trace=False):
    from concourse.bass_utils import run_bass_kernel_spmd

    nc = _get_nc()
    in_maps = _prep_inputs(inputs)
    res = run_bass_kernel_spmd(nc, in_maps, list(range(NCORES)), trace=trace)
    out = np.concatenate(
        [np.asarray(res.results[c]["out"]).T for c in range(NCORES)], axis=0)
    return out.astype(np.float32), res


def kernel(**inputs):
    out, _ = _run(inputs, trace=False)
    return out


# revision 15
# speedup vs baseline: 1.0262x; 1.0262x over previous
"""CTBG circuit kernel for Trainium2, data-parallel over batch on 8 NeuronCores.

Network (per reference):
  gpe_out = x @ (gpe_w * gpe_mask.T) + gpe_b              [B, 1536]
  gpi_in  = concat([x, gpe_out], -1)                      [B, 3072]
  gpi_out = gpi_in @ (gpi_w * gpi_mask.T) + gpi_b         [B, 3072] @ [3072, 1536]
  h1 = relu(gpi_out @ w1 + b1); h2 = relu(h1 @ w2 + b2)
  out = relu(h2 @ w3 + b3)                                [B, 6]

Key algebraic identity: gpe_out and gpi_out feed forward with no
intervening nonlinearity, so the masked front end folds into one
[1536, 512] weight computed ON DEVICE once per launch:

  A  = gpe_w * gpe_mask.T          [1536 i, 1536 u]
  Bx = (gpi_w * gpi_mask.T)[:1536] [1536 i, 1536 v]
  Bu = (gpi_w * gpi_mask.T)[1536:] [1536 u, 1536 v]
  Wfold = Bx @ w1 + A @ (Bu @ w1)  [1536, 512]
  bfold = gpe_b @ (Bu @ w1) + gpi_b @ w1 + b1
  h1 = relu(x @ Wfold + bfold) -> h2 -> out   (per batch row)

Distribution: a fixed ~36-45us collectives-init barrier on this
platform gates the FIRST collective completion to ~90us into the
launch, so chained collectives (gather M, then gather Wfold) are
poison.  Instead every core computes a full-shape PARTIAL of Wfold
from purely local slices, and ONE AllReduce(add) sums them:

  core c:  M_c = Bu[usl_c] @ w1                 [192, 512]  (local)
           P_c = Bx[:, vsl_c] @ w1[vsl_c]       [1536, 512] (partial
               + A[:, usl_c] @ M_c                            sums)
           prow_c = gpe_b[usl_c] @ M_c + gpi_b[vsl_c] @ w1[vsl_c]
  AllReduce over cores: Wfold = sum_c P_c ; bias row = sum_c prow_c.

The AllReduce is split into two h-halves so the batch pass starts on
h-columns 0:256 while the second half is still on the wire.  The
batch pass does hc 0/1 across all 4 batch tiles (stationary reused,
gated on AR half a only), then per-tile [hc2, hc3, L2, L3 + store] so
each output store trails its own tile.  All bulk tensors move as one
large DMA each (rearranged [p, chunk, col] APs) because per-DMA issue
overhead (~0.6-1us) otherwise dominates the fold.  gpsimd carries
only the collectives; sync carries the dependency-laden loads; scalar
carries free-flowing streams.

Host prep is layout/dtype only (no FLOPs): bf16 casts, transposes of
x/gpe_w/gpi_w, row/column slicing and an even/odd interleave
permutation of each 192-row slice (so the two 96-row PE groups are
contiguous and drains are single DMAs).
"""

import numpy as np
import ml_dtypes

BF = ml_dtypes.bfloat16

NCORES = 8
B = 16384
BS = B // NCORES          # 2048 rows per core
BT = 512                  # batch tile (matmul free dim)
NBT = BS // BT            # 4
D1 = 1536                 # gpe input dim (x features)
H = 512                   # mlp hidden
HH = H // 2               # 256: AllReduce column half
A = 6                     # action dim
SL = D1 // NCORES         # 192: fold rows per core
HSL = SL // 2             # 96: interleaved half-slice
PR = D1 + 1               # AllReduce rows: 1536 Wfold + 1 bias row

NI = D1 // 128            # 12 i-chunks (x features)
NV = D1 // 128            # 12 v-chunks (gpi outputs)
NH = H // 128             # 4 h-chunks (mlp hidden)

_CACHE = {}


def _build():
    import concourse.bacc as bacc
    import concourse.tile as tile
    from concourse import mybir
    from concourse.masks import make_identity

    FP32 = mybir.dt.float32
    BF16 = mybir.dt.bfloat16
    Act = mybir.ActivationFunctionType

    nc = bacc.Bacc(None, num_devices=NCORES)

    xT_d = nc.dram_tensor("xT", [D1, BS], BF16, kind="ExternalInput")
    # [1536, 384] = [masked uslice cols | wT uslice cols], interleave-permuted
    gpiu_d = nc.dram_tensor("gpiu", [D1, 2 * SL], BF16, kind="ExternalInput")
    # [192, 3072] = [mask | wT] rows vsl (BxT) / usl (AT), interleave-permuted
    bxp_d = nc.dram_tensor("bxp", [SL, 2 * D1], BF16, kind="ExternalInput")
    ap_d = nc.dram_tensor("apk", [SL, 2 * D1], BF16, kind="ExternalInput")
    w1_d = nc.dram_tensor("w1", [D1, H], BF16, kind="ExternalInput")
    w1vs_d = nc.dram_tensor("w1vs", [SL, H], BF16, kind="ExternalInput")
    w2_d = nc.dram_tensor("w2", [H, H], BF16, kind="ExternalInput")
    w3_d = nc.dram_tensor("w3", [H, A], BF16, kind="ExternalInput")
    gpebp_d = nc.dram_tensor("gpebp", [HSL, 2], FP32, kind="ExternalInput")
    gpibp_d = nc.dram_tensor("gpibp", [HSL, 2], FP32, kind="ExternalInput")
    b1_d = nc.dram_tensor("b1", [H], FP32, kind="ExternalInput")
    b2_d = nc.dram_tensor("b2", [H], FP32, kind="ExternalInput")
    b3_d = nc.dram_tensor("b3", [A], FP32, kind="ExternalInput")
    o_d = nc.dram_tensor("out", [A, BS], FP32, kind="ExternalOutput")

    RG = [list(range(NCORES))]
    SLW = 2 * SL              # 384: packed gpiu width per v-chunk

    with tile.TileContext(nc) as tc:
        with (
            tc.tile_pool(name="wp", bufs=1) as wp,           # persistent
            tc.tile_pool(name="ap", bufs=1) as ap,           # activations
            tc.tile_pool(name="dp", bufs=1, space="DRAM") as dp,
            tc.tile_pool(name="psp", bufs=8, space="PSUM") as psp,
        ):
            def ps_tile():
                return psp.tile([128, BT], FP32, tag="ps", name="ps")

            # ---- bulk fold loads, one DMA each
            gpiu = wp.tile([128, NV * SLW], BF16, tag="gpiu")
            nc.sync.dma_start(
                out=gpiu[:, :].rearrange("p (v c) -> p v c", v=NV),
                in_=gpiu_d.rearrange("(v p) c -> p v c", p=128))
            w1a = wp.tile([128, NV * H], BF16, tag="w1a")
            nc.scalar.dma_start(
                out=w1a[:, :].rearrange("p (v c) -> p v c", v=NV),
                in_=w1_d.rearrange("(v p) c -> p v c", p=128))
            for v in range(NV):
                nc.vector.tensor_mul(gpiu[:, v * SLW:v * SLW + SL],
                                     gpiu[:, v * SLW:v * SLW + SL],
                                     gpiu[:, v * SLW + SL:v * SLW + 2 * SL])

            bxp, apk, w1vs = [], [], []
            for g in range(2):
                t = wp.tile([HSL, 2 * D1], BF16, tag=f"bxp{g}")
                nc.sync.dma_start(out=t[:, :],
                                  in_=bxp_d[g * HSL:(g + 1) * HSL, :])
                nc.vector.tensor_mul(t[:, 0:D1], t[:, 0:D1], t[:, D1:2 * D1])
                bxp.append(t)
                t = wp.tile([HSL, 2 * D1], BF16, tag=f"apk{g}")
                nc.scalar.dma_start(out=t[:, :],
                                    in_=ap_d[g * HSL:(g + 1) * HSL, :])
                nc.vector.tensor_mul(t[:, 0:D1], t[:, 0:D1], t[:, D1:2 * D1])
                apk.append(t)
                t = wp.tile([HSL, H], BF16, tag=f"w1vs{g}")
                nc.sync.dma_start(out=t[:, :],
                                  in_=w1vs_d[g * HSL:(g + 1) * HSL, :])
                w1vs.append(t)

            # small loads
            gpebp = wp.tile([HSL, 2], FP32, tag="gpebp")
            nc.scalar.dma_start(out=gpebp[:, :], in_=gpebp_d[:, :])
            gpibp = wp.tile([HSL, 2], FP32, tag="gpibp")
            nc.scalar.dma_start(out=gpibp[:, :], in_=gpibp_d[:, :])
            gpebf = wp.tile([HSL, 2], BF16, tag="gpebf")
            nc.vector.tensor_copy(gpebf[:, :], gpebp[:, :])
            gpibf = wp.tile([HSL, 2], BF16, tag="gpibf")
            nc.vector.tensor_copy(gpibf[:, :], gpibp[:, :])
            b2_sb = wp.tile([128, NH], FP32, tag="b2sb")
            nc.scalar.dma_start(out=b2_sb[:, :],
                                in_=b2_d.rearrange("(c p) -> p c", p=128))
            b3_sb = wp.tile([A, 1], FP32, tag="b3sb")
            nc.scalar.dma_start(out=b3_sb[:, :],
                                in_=b3_d.rearrange("(a one) -> a one", one=1))
            b1row = wp.tile([1, H], FP32, tag="b1row")
            nc.scalar.dma_start(out=b1row[:, :],
                                in_=b1_d.rearrange("(one h) -> one h", one=1))
            w2a = wp.tile([128, NH * H], BF16, tag="w2a")
            nc.scalar.dma_start(
                out=w2a[:, :].rearrange("p (k c) -> p k c", k=NH),
                in_=w2_d.rearrange("(k p) c -> p k c", p=128))
            w3t = []
            for k in range(NH):
                t = wp.tile([128, A], BF16, tag=f"w3_{k}")
                nc.scalar.dma_start(out=t[:, :], in_=w3_d[k * 128:(k + 1) * 128, :])
                w3t.append(t)
            ident = wp.tile([128, 128], FP32, tag="ident")
            make_identity(nc, ident[:, :])

            # ---- F1s: local M slice, two interleaved 96-row groups ->
            # msb[:, g*512:(g+1)*512] holds M rows {2p+g} in bf16
            ps_m = [ps_tile() for _ in range(2)]
            for v in range(NV):
                for g in range(2):
                    nc.tensor.matmul(ps_m[g][0:HSL, :],
                                     gpiu[:, v * SLW + g * HSL:
                                          v * SLW + (g + 1) * HSL],
                                     w1a[:, v * H:(v + 1) * H],
                                     start=(v == 0), stop=(v == NV - 1))
            msb = wp.tile([HSL, 2 * H], BF16, tag="msb")
            for g in range(2):
                nc.vector.tensor_copy(msb[:, g * H:(g + 1) * H],
                                      ps_m[g][0:HSL, :])

            # ---- partial P chunks into one staging tile, two bulk stores
            pall = wp.tile([128, NI * BT], BF16, tag="pall")
            pa_dram = dp.tile([PR, HH], BF16, tag="pa_d")
            pb_dram = dp.tile([PR, HH], BF16, tag="pb_d")
            for i in range(NI):
                ps = ps_tile()
                for g in range(2):
                    nc.tensor.matmul(ps[:, :],
                                     bxp[g][:, i * 128:(i + 1) * 128],
                                     w1vs[g][:, :],
                                     start=(g == 0), stop=False)
                for g in range(2):
                    nc.tensor.matmul(ps[:, :],
                                     apk[g][:, i * 128:(i + 1) * 128],
                                     msb[:, g * H:(g + 1) * H],
                                     start=False, stop=(g == 1))
                nc.vector.tensor_copy(pall[:, i * BT:(i + 1) * BT], ps[:, :])
            pv = pall[:, :].rearrange("p (i c) -> p i c", i=NI)
            nc.sync.dma_start(
                out=pa_dram[0:D1, :].rearrange("(i p) h -> p i h", p=128),
                in_=pv[:, :, 0:HH])
            nc.scalar.dma_start(
                out=pb_dram[0:D1, :].rearrange("(i p) h -> p i h", p=128),
                in_=pv[:, :, HH:2 * HH])

            # bias partial row (row 1536 of the AllReduce payload)
            psb = ps_tile()
            for g in range(2):
                nc.tensor.matmul(psb[0:1, :], gpibf[:, g:g + 1], w1vs[g][:, :],
                                 start=(g == 0), stop=False)
            for g in range(2):
                nc.tensor.matmul(psb[0:1, :], gpebf[:, g:g + 1],
                                 msb[:, g * H:(g + 1) * H],
                                 start=False, stop=(g == 1))
            prow = wp.tile([1, H], BF16, tag="prow")
            nc.vector.tensor_copy(prow[:, :], psb[0:1, :])
            nc.sync.dma_start(out=pa_dram[D1:PR, :], in_=prow[:, 0:HH])
            nc.scalar.dma_start(out=pb_dram[D1:PR, :], in_=prow[:, HH:2 * HH])

            # ---- ONE AllReduce, split into two h-halves
            wfa_dram = dp.tile([PR, HH], BF16, tag="wfa_d", addr_space="Shared")
            wfb_dram = dp.tile([PR, HH], BF16, tag="wfb_d", addr_space="Shared")
            nc.gpsimd.collective_compute(
                "AllReduce", mybir.AluOpType.add, replica_groups=RG,
                ins=[pa_dram[:, :].opt()], outs=[wfa_dram[:, :].opt()])
            nc.gpsimd.collective_compute(
                "AllReduce", mybir.AluOpType.add, replica_groups=RG,
                ins=[pb_dram[:, :].opt()], outs=[wfb_dram[:, :].opt()])

            # ---- x tiles stream on scalar meanwhile, one DMA per batch tile
            xall = []
            for t_i in range(NBT):
                t = wp.tile([128, NI * BT], BF16, tag=f"x{t_i}")
                nc.scalar.dma_start(
                    out=t[:, :].rearrange("p (i c) -> p i c", i=NI),
                    in_=xT_d[:, t_i * BT:(t_i + 1) * BT].rearrange(
                        "(i p) c -> p i c", p=128))
                xall.append(t)

            # ---- Wfold reloads: bias row + bulk half per AllReduce
            browb = wp.tile([1, H], BF16, tag="browb")
            nc.sync.dma_start(out=browb[:, 0:HH], in_=wfa_dram[D1:PR, :])
            wfh = []
            for half, src in enumerate((wfa_dram, wfb_dram)):
                t = wp.tile([128, NI * HH], BF16, tag=f"Wf{half}")
                if half == 1:
                    nc.sync.dma_start(out=browb[:, HH:H], in_=src[D1:PR, :])
                nc.sync.dma_start(
                    out=t[:, :].rearrange("p (i c) -> p i c", i=NI),
                    in_=src[0:D1, :].rearrange("(i p) h -> p i h", p=128))
                wfh.append(t)

            def wf_sl(hc, i):
                return wfh[hc // 2][:, i * HH + (hc % 2) * 128:
                                    i * HH + (hc % 2 + 1) * 128]

            # bias row + b1, transposed [1,512] -> [128,4] columns on the PE
            # (idle right after each AR half lands), per half so hc 0/1
            # activations don't wait on AR b.
            brow = wp.tile([1, H], FP32, tag="brow")
            bfold = wp.tile([128, NH], FP32, tag="bfold")

            def bias_half(half):
                lo, hi = half * HH, (half + 1) * HH
                nc.vector.tensor_add(brow[:, lo:hi], browb[:, lo:hi],
                                     b1row[:, lo:hi])
                for c in range(2 * half, 2 * half + 2):
                    pst = ps_tile()
                    nc.tensor.transpose(pst[:, 0:1],
                                        brow[0:1, c * 128:(c + 1) * 128],
                                        ident[0:1, 0:1])
                    nc.vector.tensor_copy(bfold[:, c:c + 1], pst[:, 0:1])

            bias_half(0)

            # ---- batch pass: hc 0/1 across all 4 batch tiles (gated on AR a
            # only), then per-tile [hc2, hc3, L2, L3 + store] so each output
            # store trails its own tile instead of the whole batch
            h1 = [[None] * NH for _ in range(NBT)]
            for hc in range(2):
                ps1 = [ps_tile() for _ in range(NBT)]
                for i in range(NI):
                    for t_i in range(NBT):
                        nc.tensor.matmul(ps1[t_i][:, :], wf_sl(hc, i),
                                         xall[t_i][:, i * BT:(i + 1) * BT],
                                         start=(i == 0), stop=(i == NI - 1))
                for t_i in range(NBT):
                    h = ap.tile([128, BT], BF16, tag=f"h1_{t_i}_{hc}")
                    nc.scalar.activation(h[:, :], ps1[t_i][:, :], Act.Relu,
                                         bias=bfold[:, hc:hc + 1])
                    h1[t_i][hc] = h

            bias_half(1)

            for t_i in range(NBT):
                for hc in range(2, NH):
                    ps1 = ps_tile()
                    for i in range(NI):
                        nc.tensor.matmul(ps1[:, :], wf_sl(hc, i),
                                         xall[t_i][:, i * BT:(i + 1) * BT],
                                         start=(i == 0), stop=(i == NI - 1))
                    h = ap.tile([128, BT], BF16, tag=f"h1_{t_i}_{hc}")
                    nc.scalar.activation(h[:, :], ps1[:, :], Act.Relu,
                                         bias=bfold[:, hc:hc + 1])
                    h1[t_i][hc] = h

                h2 = []
                for mc in range(NH):
                    ps2 = ps_tile()
                    for k in range(NH):
                        nc.tensor.matmul(ps2[:, :],
                                         w2a[:, k * H + mc * 128:
                                             k * H + (mc + 1) * 128],
                                         h1[t_i][k][:, :],
                                         start=(k == 0), stop=(k == NH - 1))
                    h = ap.tile([128, BT], BF16, tag=f"h2_{t_i}_{mc}")
                    nc.scalar.activation(h[:, :], ps2[:, :], Act.Relu,
                                         bias=b2_sb[:, mc:mc + 1])
                    h2.append(h)

                pso = ps_tile()
                for k in range(NH):
                    nc.tensor.matmul(pso[0:A, :], w3t[k][:, :], h2[k][:, :],
                                     start=(k == 0), stop=(k == NH - 1))
                osb = ap.tile([A, BT], FP32, tag=f"osb{t_i}")
                nc.scalar.activation(osb[:, :], pso[0:A, :], Act.Relu,
                                     bias=b3_sb[:, 0:1])
                nc.sync.dma_start(out=o_d[:, t_i * BT:(t_i + 1) * BT],
                                  in_=osb[:, :])

    nc.finalize()
    return nc


def _get_nc():
    if "nc" not in _CACHE:
        _CACHE["nc"] = _build()
    return _CACHE["nc"]


def _prep_inputs(inputs):
    """Host-side layout/dtype prep only (no network FLOPs): bf16 casts,
    transposes, per-core row/column slicing and interleave permutation."""
    f = {k: np.asarray(v) for k, v in inputs.items()}
    xT = np.ascontiguousarray(f["x"].astype(BF).T)            # [1536, B]
    gpem = f["gpe_mask"].astype(BF)                           # [u, i]
    gpewT = np.ascontiguousarray(f["gpe_w"].astype(BF).T)     # [u, i]
    gpim = f["gpi_mask"].astype(BF)                           # [v, j]
    gpiwT = np.ascontiguousarray(f["gpi_w"].astype(BF).T)     # [v, j]
    w1 = f["w1"].astype(BF)
    gpe_b = np.asarray(f["gpe_b"], dtype=np.float32)
    gpi_b = np.asarray(f["gpi_b"], dtype=np.float32)
    # packed position g*96 + p  <->  logical slice index 2p + g
    perm = np.concatenate([np.arange(0, SL, 2), np.arange(1, SL, 2)])
    shared = {
        "w1": np.ascontiguousarray(w1),
        "w2": np.ascontiguousarray(f["w2"].astype(BF)),
        "w3": np.ascontiguousarray(f["w3"].astype(BF)),
        "b1": np.ascontiguousarray(f["b1"], dtype=np.float32),
        "b2": np.ascontiguousarray(f["b2"], dtype=np.float32),
        "b3": np.ascontiguousarray(f["b3"], dtype=np.float32),
    }
    in_maps = []
    for c in range(NCORES):
        sl = np.arange(c * SL, (c + 1) * SL)[perm]   # permuted local slice
        usl = D1 + sl                                # gpi columns for u-part
        in_maps.append(dict(
            shared,
            xT=np.ascontiguousarray(xT[:, c * BS:(c + 1) * BS]),
            gpiu=np.ascontiguousarray(
                np.concatenate([gpim[:, usl], gpiwT[:, usl]], axis=1)),
            bxp=np.ascontiguousarray(
                np.concatenate([gpim[sl][:, :D1], gpiwT[sl][:, :D1]], axis=1)),
            apk=np.ascontiguousarray(
                np.concatenate([gpem[sl], gpewT[sl]], axis=1)),
            w1vs=np.ascontiguousarray(w1[sl]),
            gpebp=np.ascontiguousarray(
                gpe_b[sl].reshape(2, HSL).T, dtype=np.float32),
            gpibp=np.ascontiguousarray(
                gpi_b[sl].reshape(2, HSL).T, dtype=np.float32),
        ))
    return in_maps


def _run(inputs, trace=False):
    from concourse.bass_utils import run_bass_kernel_spmd

    nc = _get_nc()
    in_maps = _prep_inputs(inputs)
    res = run_bass_kernel_spmd(nc, in_maps, list(range(NCORES)), trace=trace)
    out = np.concatenate(
        [np.asarray(res.results[c]["out"]).T for c in range(NCORES)], axis=0)
    return out.astype(np.float32), res


def kernel(**inputs):
    out, _ = _run(inputs, trace=False)
    return out
